# revision 19
# baseline (speedup 1.0000x reference)
"""Trainium2 Bass kernel for nn_EquiConv (e3nn-style FullyConnectedTensorProduct
+ gate + radial-MLP elementwise conv), data-parallel over edges on 8 cores.

Host side: shard edges, pad, pre-transpose / pre-convert inputs, pre-arrange
weight tensors into contraction-major (k-major) chunk layout.

Device side (per core, per 128-edge tile):
  - DVE builds the per-edge outer-product activations edge-major (broadcast APs)
  - PE transposes them to k-major (contraction on partitions)
  - ACT copies PSUM->SBUF (bf16)
  - PE accumulates chunked matmuls against shared weights into PSUM
  - MLP (3 matmuls + Silu on ACT), gate + elementwise-conv fused feature-major
Output is written feature-major and reassembled on the host.
"""

import sys

sys.path.insert(0, "/opt/trn_rl_repo")

import numpy as np
import ml_dtypes

import concourse.bass as bass
import concourse.bacc as bacc
import concourse.mybir as mybir
import concourse.tile as tile
from concourse.bass_utils import run_bass_kernel_spmd

BF16 = ml_dtypes.bfloat16

E = 20000
S = 64
V = 32
FC_IN = 128
HID = 64
INV_SQRT3 = 0.5773502691896258

NCORES = 8
EC = E // NCORES  # 2500 edges per core
ET = 128  # edges per tile
NT = (EC + ET - 1) // ET  # 20 tiles
EPAD = NT * ET  # 2560

A_SC = float(1.0 / np.sqrt(np.float32(S * S + V * V)))
A_VEC = float(1.0 / np.sqrt(np.float32(2 * S * V)))

f32 = mybir.dt.float32
bf16 = mybir.dt.bfloat16

# chunk counts along each contraction dim (each chunk is 128 wide)
N_SS = (S * S) // 128  # 32 ss chunks   -> 96-wide out (sc|g)
N_VV = (V * V) // 128  # 8 vv chunks    -> same out
N_SV = (S * V) // 128  # 16 chunks per i (sv)
N_VS = (V * S) // 128  # 16 chunks per i (vs)


def _prep_weights(w_ss_s, w_vv_s, w_ss_g, w_vv_g, w_sv_v, w_vs_v,
                  fc_w1, fc_b1, fc_w2, fc_b2, fc_w3, fc_b3):
    """Host-side rearrangement of the shared weights into k-major chunk layout."""
    wss = np.concatenate([w_ss_s, w_ss_g], axis=2) * A_SC  # [64,64,96]
    wvv = np.concatenate([w_vv_s, w_vv_g], axis=2) * (A_SC * INV_SQRT3)  # [32,32,96]
    w_ssvv = np.concatenate(
        [wss.reshape(S * S, S + V), wvv.reshape(V * V, S + V)], axis=0
    )  # [5120, 96];  k = u*64+v (ss) ++ 4096 + u*32+v (vv)
    w_ssvv = (
        w_ssvv.reshape(N_SS + N_VV, 128, S + V).transpose(1, 0, 2)
        .reshape(128, (N_SS + N_VV) * (S + V))
    )

    wsv = (w_sv_v * A_VEC).reshape(S * V, V)  # k = u*32+v
    wsv = wsv.reshape(N_SV, 128, V).transpose(1, 0, 2).reshape(128, N_SV * V)
    wvs = (w_vs_v * A_VEC).reshape(V * S, V)  # k = u*64+v (u in V, v in S)
    wvs = wvs.reshape(N_VS, 128, V).transpose(1, 0, 2).reshape(128, N_VS * V)

    sel3 = np.zeros((V, 3 * V), dtype=np.float32)  # replicate [32] -> [(i,w)=96]
    for i in range(3):
        for w in range(V):
            sel3[w, i * V + w] = 1.0

    return {
        "w_ssvv": w_ssvv.astype(BF16),
        "w_sv": wsv.astype(BF16),
        "w_vs": wvs.astype(BF16),
        "w_fc1": fc_w1.astype(BF16),  # [128, 64]
        "w_fc2": fc_w2.astype(BF16),  # [64, 64]
        "w_fc3": fc_w3.astype(BF16),  # [64, 96]
        "b_fc1": fc_b1.reshape(HID, 1).astype(np.float32),
        "b_fc2": fc_b2.reshape(HID, 1).astype(np.float32),
        "b_fc3": fc_b3.reshape(S + V, 1).astype(np.float32),
        "sel3": sel3.astype(BF16),  # [32, 96]
        "ident": np.eye(128, dtype=BF16),
    }


def _build_program(sim_compat=False):
    import os
    ABL_NO_MM = bool(int(os.environ.get("ABL_NO_MM", "0")))
    ABL_NO_TRANS = bool(int(os.environ.get("ABL_NO_TRANS", "0")))
    ABL_NO_BUILD = bool(int(os.environ.get("ABL_NO_BUILD", "0")))
    """Build the Bass/Tile program (identical on all cores)."""
    nc = bacc.Bacc("TRN2", target_bir_lowering=False, debug=False)

    d_fea1 = nc.dram_tensor("fea1", [EPAD, 160], f32, kind="ExternalInput").ap()
    d_fea2 = nc.dram_tensor("fea2", [EPAD, 160], f32, kind="ExternalInput").ap()
    d_fwT = nc.dram_tensor("fwT", [FC_IN, EPAD], bf16, kind="ExternalInput").ap()
    d_wssvv = nc.dram_tensor("w_ssvv", [128, (N_SS + N_VV) * (S + V)], bf16, kind="ExternalInput").ap()
    d_wsv = nc.dram_tensor("w_sv", [128, N_SV * V], bf16, kind="ExternalInput").ap()
    d_wvs = nc.dram_tensor("w_vs", [128, N_VS * V], bf16, kind="ExternalInput").ap()
    d_wfc1 = nc.dram_tensor("w_fc1", [FC_IN, HID], bf16, kind="ExternalInput").ap()
    d_wfc2 = nc.dram_tensor("w_fc2", [HID, HID], bf16, kind="ExternalInput").ap()
    d_wfc3 = nc.dram_tensor("w_fc3", [HID, S + V], bf16, kind="ExternalInput").ap()
    d_bfc1 = nc.dram_tensor("b_fc1", [HID, 1], f32, kind="ExternalInput").ap()
    d_bfc2 = nc.dram_tensor("b_fc2", [HID, 1], f32, kind="ExternalInput").ap()
    d_bfc3 = nc.dram_tensor("b_fc3", [S + V, 1], f32, kind="ExternalInput").ap()
    d_sel3 = nc.dram_tensor("sel3", [V, 3 * V], bf16, kind="ExternalInput").ap()
    d_ident = nc.dram_tensor("ident", [128, 128], bf16, kind="ExternalInput").ap()

    d_osc = nc.dram_tensor("out_sc", [S, EPAD], f32, kind="ExternalOutput").ap()
    d_ovec = nc.dram_tensor("out_vec", [3 * V, EPAD], f32, kind="ExternalOutput").ap()

    SiLU = mybir.ActivationFunctionType.Silu
    Sigm = mybir.ActivationFunctionType.Sigmoid
    Copy = mybir.ActivationFunctionType.Copy
    Ident = mybir.ActivationFunctionType.Identity
    mul_op = mybir.AluOpType.mult
    add_op = mybir.AluOpType.add

    def emit_silu(nc, postp, out_tile, in_ap, bias, tag):
        if not sim_compat:
            if bias is None:
                nc.scalar.activation(out_tile[:], in_ap, SiLU)
            else:
                nc.scalar.activation(out_tile[:], in_ap, SiLU, bias=bias)
            return
        shp = [out_tile[:].shape[0], out_tile[:].shape[1]]
        xm = postp.tile(shp, bf16, tag=tag + "_xm", name=tag + "_xm")
        sg = postp.tile(shp, bf16, tag=tag + "_sg", name=tag + "_sg")
        if bias is None:
            nc.scalar.activation(xm[:], in_ap, Copy)
            nc.scalar.activation(sg[:], in_ap, Sigm)
        else:
            nc.scalar.activation(xm[:], in_ap, Ident, bias=bias)
            nc.scalar.activation(sg[:], in_ap, Sigm, bias=bias)
        nc.vector.tensor_tensor(out_tile[:], xm[:], sg[:], mul_op)

    with tile.TileContext(nc) as tc:
        with (
            tc.tile_pool(name="consts", bufs=1) as consts,
            tc.tile_pool(name="io", bufs=2) as io,
            tc.tile_pool(name="kron", bufs=1) as kronp,
            tc.tile_pool(name="stage", bufs=3) as stagep,
            tc.tile_pool(name="post", bufs=2) as postp,
            tc.tile_pool(name="pst", bufs=3, space=bass.MemorySpace.PSUM) as pst,
            tc.tile_pool(name="pacc", bufs=1, space=bass.MemorySpace.PSUM) as pacc,
            tc.tile_pool(name="pmlp", bufs=2, space=bass.MemorySpace.PSUM) as pmlp,
            tc.tile_pool(name="prep", bufs=2, space=bass.MemorySpace.PSUM) as prep,
        ):
            # ---- constants (resident) ----
            wssvv = consts.tile([128, (N_SS + N_VV) * (S + V)], bf16)
            nc.sync.dma_start(wssvv[:], d_wssvv)
            wsv = consts.tile([128, N_SV * V], bf16)
            nc.sync.dma_start(wsv[:], d_wsv)
            wvs = consts.tile([128, N_VS * V], bf16)
            nc.sync.dma_start(wvs[:], d_wvs)
            wfc1 = consts.tile([FC_IN, HID], bf16)
            nc.sync.dma_start(wfc1[:], d_wfc1)
            wfc2 = consts.tile([HID, HID], bf16)
            nc.sync.dma_start(wfc2[:], d_wfc2)
            wfc3 = consts.tile([HID, S + V], bf16)
            nc.sync.dma_start(wfc3[:], d_wfc3)
            bfc1 = consts.tile([HID, 1], f32)
            nc.sync.dma_start(bfc1[:], d_bfc1)
            bfc2 = consts.tile([HID, 1], f32)
            nc.sync.dma_start(bfc2[:], d_bfc2)
            bfc3 = consts.tile([S + V, 1], f32)
            nc.sync.dma_start(bfc3[:], d_bfc3)
            sel3 = consts.tile([V, 3 * V], bf16)
            nc.sync.dma_start(sel3[:], d_sel3)
            ident = consts.tile([128, 128], bf16)
            nc.sync.dma_start(ident[:], d_ident)

            for t in range(NT):
                e0 = t * ET
                # ---- inputs ----
                fea1 = io.tile([ET, 160], f32, tag="fea1")
                nc.sync.dma_start(fea1[:], d_fea1[e0:e0 + ET, :])
                fea2 = io.tile([ET, 160], f32, tag="fea2")
                nc.sync.dma_start(fea2[:], d_fea2[e0:e0 + ET, :])
                fwT = io.tile([FC_IN, ET], bf16, tag="fwT")
                nc.sync.dma_start(fwT[:], d_fwT[:, e0:e0 + ET])

                x1 = io.tile([ET, 160], bf16, tag="x1")
                nc.vector.tensor_copy(x1[:], fea1[:])
                x2 = io.tile([ET, 160], bf16, tag="x2")
                nc.vector.tensor_copy(x2[:], fea2[:])

                x1s = x1[:, 0:S]                                    # [e, 64]
                x2s = x2[:, 0:S]
                x1v = x1[:, S:160].rearrange("e (u i) -> e u i", i=3)  # [e, 32, 3]
                x2v = x2[:, S:160].rearrange("e (u i) -> e u i", i=3)

                # ---- MLP (feature-major) ----
                h1p = pmlp.tile([S + V, ET], f32, tag="mlp")
                nc.tensor.matmul(h1p[0:HID, :], wfc1[:], fwT[:], start=True, stop=True)
                h1 = postp.tile([HID, ET], bf16, tag="h1")
                emit_silu(nc, postp, h1, h1p[0:HID, :], bfc1[:, 0:1], f"h1s{t}")
                h2p = pmlp.tile([S + V, ET], f32, tag="mlp")
                nc.tensor.matmul(h2p[0:HID, :], wfc2[:], h1[:], start=True, stop=True)
                h2 = postp.tile([HID, ET], bf16, tag="h2")
                emit_silu(nc, postp, h2, h2p[0:HID, :], bfc2[:, 0:1], f"h2s{t}")
                wp = pmlp.tile([S + V, ET], f32, tag="mlp")
                nc.tensor.matmul(wp[:], wfc3[:], h2[:], start=True, stop=True)
                wgt_sc = postp.tile([S, ET], bf16, tag="wgt_sc")
                nc.scalar.activation(wgt_sc[:], wp[0:S, :], Ident, bias=bfc3[0:S, 0:1])
                wgt_v = postp.tile([V, ET], bf16, tag="wgt_v")
                nc.scalar.activation(wgt_v[:], wp[S:S + V, :], Ident, bias=bfc3[S:S + V, 0:1])

                # ---- DVE: edge-major outer products (bf16) ----
                # kron_ss [e, u*64+v] = x1s[u] * x2s[v]
                kss = kronp.tile([ET, S * S], bf16, tag="kss")
                nc.vector.tensor_tensor(
                    kss[:].rearrange("e (u v) -> e u v", v=S),
                    x1s.unsqueeze(2).broadcast_to([ET, S, S]),
                    x2s.unsqueeze(1).broadcast_to([ET, S, S]),
                    mul_op,
                )

                # dot_vv [e, u*32+v] = sum_i x1v[u,i] * x2v[v,i]
                pv = [kronp.tile([ET, V * V], bf16, tag=f"pv{i}", name=f"pv{i}_{t}") for i in range(3)]
                for i in range(3):
                    nc.vector.tensor_tensor(
                        pv[i][:].rearrange("e (u v) -> e u v", v=V),
                        x1v[:, :, i].unsqueeze(2).broadcast_to([ET, V, V]),
                        x2v[:, :, i].unsqueeze(1).broadcast_to([ET, V, V]),
                        mul_op,
                    )
                kvv = kronp.tile([ET, V * V], bf16, tag="kvv")
                nc.vector.tensor_tensor(kvv[:], pv[0][:], pv[1][:], add_op)
                nc.vector.tensor_tensor(kvv[:], kvv[:], pv[2][:], add_op)

                # kron_sv_i [e, u*32+v] = x1s[u] * x2v[v,i]
                ksv = [kronp.tile([ET, S * V], bf16, tag=f"ksv{i}", name=f"ksv{i}_{t}") for i in range(3)]
                for i in range(3):
                    nc.vector.tensor_tensor(
                        ksv[i][:].rearrange("e (u v) -> e u v", v=V),
                        x1s.unsqueeze(2).broadcast_to([ET, S, V]),
                        x2v[:, :, i].unsqueeze(1).broadcast_to([ET, S, V]),
                        mul_op,
                    )
                # kron_vs_i [e, u*64+v] = x1v[u,i] * x2s[v]
                kvs = [kronp.tile([ET, V * S], bf16, tag=f"kvs{i}", name=f"kvs{i}_{t}") for i in range(3)]
                for i in range(3):
                    nc.vector.tensor_tensor(
                        kvs[i][:].rearrange("e (u v) -> e u v", v=S),
                        x1v[:, :, i].unsqueeze(2).broadcast_to([ET, V, S]),
                        x2s.unsqueeze(1).broadcast_to([ET, V, S]),
                        mul_op,
                    )

                # ---- transpose chunks to k-major, accumulate main matmuls ----
                acc_ss = pacc.tile([S + V, ET], f32, tag="acc_ss")
                acc_v = pacc.tile([3 * V, ET], f32, tag="acc_v")

                mm_list = []  # (kron, k-off, wtile, w-off, acc, row0, width, first, last)
                for c in range(N_SS):
                    mm_list.append((kss, c * 128, wssvv, c * (S + V), acc_ss, 0, S + V,
                                    c == 0, False))
                for c in range(N_VV):
                    mm_list.append((kvv, c * 128, wssvv, (N_SS + c) * (S + V), acc_ss,
                                    0, S + V, False, c == N_VV - 1))
                for i in range(3):
                    for c in range(N_SV):
                        mm_list.append((ksv[i], c * 128, wsv, c * V, acc_v, i * V, V,
                                        c == 0, False))
                    for c in range(N_VS):
                        mm_list.append((kvs[i], c * 128, wvs, c * V, acc_v, i * V, V,
                                        False, c == N_VS - 1))

                for (kem, koff, wt, woff, acc, r0, wid, first, last) in mm_list:
                    st = pst.tile([128, ET], bf16, tag="stage")
                    nc.tensor.transpose(st[:], kem[:, koff:koff + 128], ident[:])
                    sb = stagep.tile([128, ET], bf16, tag="stage_sb")
                    nc.scalar.activation(sb[:], st[:], Copy)
                    tp = (0, r0) if wid == V else None
                    nc.tensor.matmul(
                        acc[r0:r0 + wid, :], wt[:, woff:woff + wid], sb[:],
                        start=first, stop=last, tile_position=tp,
                    )

                # ---- gate + elementwise conv (feature-major) ----
                silu_sc = postp.tile([S, ET], bf16, tag="silu_sc")
                emit_silu(nc, postp, silu_sc, acc_ss[0:S, :], None, f"scs{t}")
                sig = postp.tile([V, ET], bf16, tag="sig")
                nc.scalar.activation(sig[:], acc_ss[S:S + V, :], Sigm)

                sigrep_p = prep.tile([3 * V, ET], f32, tag="rep")
                nc.tensor.matmul(sigrep_p[:], sel3[:], sig[:], start=True, stop=True)
                wvrep_p = prep.tile([3 * V, ET], f32, tag="rep")
                nc.tensor.matmul(wvrep_p[:], sel3[:], wgt_v[:], start=True, stop=True)
                sigrep = postp.tile([3 * V, ET], bf16, tag="sigrep")
                nc.scalar.activation(sigrep[:], sigrep_p[:], Copy)
                wvrep = postp.tile([3 * V, ET], bf16, tag="wvrep")
                nc.scalar.activation(wvrep[:], wvrep_p[:], Copy)

                osc = postp.tile([S, ET], f32, tag="osc")
                nc.gpsimd.tensor_tensor(osc[:], silu_sc[:], wgt_sc[:], mul_op)
                vg = postp.tile([3 * V, ET], bf16, tag="vg")
                nc.vector.tensor_tensor(vg[:], acc_v[:], sigrep[:], mul_op)
                ovec = postp.tile([3 * V, ET], f32, tag="ovec")
                nc.vector.tensor_tensor(ovec[:], vg[:], wvrep[:], mul_op)

                nc.sync.dma_start(d_osc[:, e0:e0 + ET], osc[:])
                nc.sync.dma_start(d_ovec[:, e0:e0 + ET], ovec[:])

    nc.compile()
    return nc


_CACHED = {}


def kernel(fea_in1, fea_in2, fea_weight,
           w_ss_s, w_vv_s, w_ss_g, w_vv_g, w_sv_v, w_vs_v,
           fc_w1, fc_b1, fc_w2, fc_b2, fc_w3, fc_b3, batch_edge):
    fea_in1 = np.asarray(fea_in1, dtype=np.float32)
    fea_in2 = np.asarray(fea_in2, dtype=np.float32)
    fea_weight = np.asarray(fea_weight, dtype=np.float32)

    wd = _prep_weights(np.asarray(w_ss_s, np.float32), np.asarray(w_vv_s, np.float32),
                       np.asarray(w_ss_g, np.float32), np.asarray(w_vv_g, np.float32),
                       np.asarray(w_sv_v, np.float32), np.asarray(w_vs_v, np.float32),
                       np.asarray(fc_w1, np.float32), np.asarray(fc_b1, np.float32),
                       np.asarray(fc_w2, np.float32), np.asarray(fc_b2, np.float32),
                       np.asarray(fc_w3, np.float32), np.asarray(fc_b3, np.float32))

    if "nc" not in _CACHED:
        _CACHED["nc"] = _build_program()
    nc = _CACHED["nc"]

    in_maps = []
    for c in range(NCORES):
        s0 = c * EC
        f1 = np.zeros((EPAD, 160), np.float32)
        f1[:EC] = fea_in1[s0:s0 + EC]
        f2 = np.zeros((EPAD, 160), np.float32)
        f2[:EC] = fea_in2[s0:s0 + EC]
        fwT = np.zeros((FC_IN, EPAD), BF16)
        fwT[:, :EC] = fea_weight[s0:s0 + EC].T.astype(BF16)
        m = {"fea1": f1, "fea2": f2, "fwT": fwT}
        m.update(wd)
        in_maps.append(m)

    import os
    trace = bool(int(os.environ.get("KERNEL_TRACE", "0")))
    res = run_bass_kernel_spmd(nc, in_maps, core_ids=list(range(NCORES)), trace=trace)
    _CACHED["exec_time_ns"] = res.exec_time_ns

    out = np.empty((E, S + 3 * V), np.float32)
    # vec partition p = i*32+w  ->  output column 64 + 3*w + i
    vec_cols = np.empty(3 * V, np.int64)
    for i in range(3):
        for w in range(V):
            vec_cols[i * V + w] = S + 3 * w + i
    for c in range(NCORES):
        s0 = c * EC
        osc = np.asarray(res.results[c]["out_sc"])[:, :EC]    # [64, EC]
        ovec = np.asarray(res.results[c]["out_vec"])[:, :EC]  # [96, EC]
        out[s0:s0 + EC, :S] = osc.T
        out[s0:s0 + EC, vec_cols] = ovec.T
    return out


if __name__ == "__main__":
    rng = np.random.default_rng(0)
    ins = {
        "fea_in1": rng.standard_normal((E, 160)).astype(np.float32),
        "fea_in2": rng.standard_normal((E, 160)).astype(np.float32),
        "fea_weight": rng.standard_normal((E, FC_IN)).astype(np.float32),
        "w_ss_s": rng.standard_normal((S, S, S)).astype(np.float32),
        "w_vv_s": rng.standard_normal((V, V, S)).astype(np.float32),
        "w_ss_g": rng.standard_normal((S, S, V)).astype(np.float32),
        "w_vv_g": rng.standard_normal((V, V, V)).astype(np.float32),
        "w_sv_v": rng.standard_normal((S, V, V)).astype(np.float32),
        "w_vs_v": rng.standard_normal((V, S, V)).astype(np.float32),
        "fc_w1": rng.standard_normal((FC_IN, HID)).astype(np.float32),
        "fc_b1": np.zeros(HID, np.float32),
        "fc_w2": rng.standard_normal((HID, HID)).astype(np.float32),
        "fc_b2": np.zeros(HID, np.float32),
        "fc_w3": rng.standard_normal((HID, S + V)).astype(np.float32),
        "fc_b3": np.zeros(S + V, np.float32),
        "batch_edge": np.zeros(E, np.int32),
    }
    out = kernel(**ins)
    print("kernel out", out.shape, out.dtype, float(np.abs(out).mean()))
